# revision 1
# baseline (speedup 1.0000x reference)
"""Trainium2 Bass kernel for nn_AFMExpert (AFM attention-over-pairs net).

Math (per batch b):
    had[p, e]  = x[b, i_p, e] * x[b, j_p, e]          P = F*(F-1)/2 pairs
    a          = relu(had @ W1^T + b1)                 [P, NHID]
    logits     = a @ W2^T + b2                         [P, OUT]
    attn       = softmax(logits, axis=pairs)
    afm[e]     = sum_p attn[p, e] * had[p, e]          (OUT == E)
    out[b]     = afm @ pw^T + pb                       [1]

Distribution: pure data parallel, batch sharded 8 ways (64 batches/core),
weights replicated.  No collectives.

Per-core layout ("L1"): two batches stacked on the 128 SBUF partitions
(partition = t*64 + e, t in {0,1}), pair dim + batch-pair dim on the free
axis.  Pairs are enumerated by rotation diagonals: d in 1..31 -> (k, (k+d)%64)
for k in 0..63, plus d=32 -> (k, k+32) for k in 0..31, so `had` generation is
32 large DVE tensor_tensor ops using 3D access patterns (no gathers).  The
pair block is padded 2016 -> 2048 so matmul chunks land exactly on PSUM
banks; the softmax normalizer is corrected for the 32 dead columns with a
host-precomputed constant (dead had cols are zero => their logits are a
data-independent constant).

Matmuls use block-diagonal [128,128] lhsT (the two stacked batches share
the PE array; one self-loading matmul per PSUM bank chunk).  The relu
evacuation runs in halves (ACT activation for most pairs, DVE
scalar_tensor_tensor for a few — an engine-balance knob) so mm2 of half h
overlaps the evac of half h+1.  mm2/exp/attn-sum run on [128,1024]
halves: exp is computed IN PLACE in PSUM (ScalarE->PSUM is the fast
direction) with the softmax normalizer Z emitted free via
activation(accum_out=); the attn*had product+reduction is one fused DVE
scalar_tensor_tensor(accum_out=) reading exp straight from PSUM.
had-generation for group g+1 is interleaved between group g's pairs so
the in-order DVE queue chews on it while ACT/PE finish each pair's deps.

Measured on 8 axon trn2 cores: ~189-190us NEFF exec, rms rel err ~1.1e-3.
"""

import os
import sys

for _p in ("/opt/trn_rl_repo", "/opt/pypackages"):
    if os.path.isdir(_p) and _p not in sys.path:
        sys.path.append(_p)

from contextlib import ExitStack

import ml_dtypes
import numpy as np

import concourse.bass as bass
import concourse.mybir as mybir
import concourse.tile as tile
from concourse import bacc
from concourse.bass_utils import run_bass_kernel_spmd

BF16 = mybir.dt.bfloat16
F32 = mybir.dt.float32

B, F, E, NHID, OUT = 512, 64, 64, 64, 64
NCORES = 8
BLOC = B // NCORES          # 64 batches per core
NPAIR = BLOC // 2           # 32 batch-pairs per core
PF = 2048                   # padded pair-block width (4 PSUM banks of f32)
NREAL = 2016                # real pairs = F*(F-1)/2
XW = 192                    # xrot row width: 96 even-base + 96 odd-base cols

# batch-pair groups for `had` generation; each group's 32 DVE ops are
# emitted just before that group's per-pair work so had-generation
# pipelines against the previous group's ACT/PE/DVE work
HAD_GROUPS = (4, 7, 7, 7, 7)
# pairs whose relu evacuation runs on DVE instead of ACT (a balance
# knob; placed late, where DVE has no had-generation left to chew on)
RELU_DVE_PAIRS = frozenset((8, 16, 24))
PH = PF // 2                # ps2 half width (2 PSUM banks)


def _build_nc():
    # Bacc (not raw Bass): its finalize() runs generate_event_semaphores,
    # which splits multi-wait sync_info onto InstEventSemaphore — the TRN2
    # ISA allows at most 1 sync wait per regular instruction.
    nc = bacc.Bacc(None)

    xrot = nc.declare_dram_parameter("xrot", [128, NPAIR * XW], BF16, isOutput=False)
    w1s_d = nc.declare_dram_parameter("w1s", [128, 128], BF16, isOutput=False)
    w2s_d = nc.declare_dram_parameter("w2s", [128, 128], BF16, isOutput=False)
    b1s_d = nc.declare_dram_parameter("b1s", [128, 1], F32, isOutput=False)
    b2s_d = nc.declare_dram_parameter("b2s", [128, 1], F32, isOutput=False)
    pws_d = nc.declare_dram_parameter("pws", [128, 1], F32, isOutput=False)
    zcorr_d = nc.declare_dram_parameter("zcorr", [128, 1], F32, isOutput=False)
    mask_d = nc.declare_dram_parameter("mask", [128, 2], F32, isOutput=False)
    pb_d = nc.declare_dram_parameter("pb", [2, 1], F32, isOutput=False)
    out_d = nc.declare_dram_parameter("out", [BLOC, 1], F32, isOutput=True)

    with tile.TileContext(nc) as tc, ExitStack() as ctx:
        const = ctx.enter_context(tc.tile_pool(name="const", bufs=1))
        hadp = ctx.enter_context(tc.tile_pool(name="hadp", bufs=1))
        ring = ctx.enter_context(tc.tile_pool(name="ring", bufs=3))
        small = ctx.enter_context(tc.tile_pool(name="small", bufs=1))
        ps1p = ctx.enter_context(tc.tile_pool(name="ps1p", bufs=1, space="PSUM"))
        ps2p = ctx.enter_context(tc.tile_pool(name="ps2p", bufs=2, space="PSUM"))

        # ---- constants / inputs to SBUF ----
        # xrot is DMA'd per had-group so group 0's had-generation can
        # start after ~0.5us instead of waiting for the full transfer
        xr_tiles = {}
        g0 = 0
        for gn in HAD_GROUPS:
            xr_g = const.tile([128, gn, XW], BF16, tag=f"xr{g0}",
                              name=f"xr{g0}")
            nc.sync.dma_start(
                out=xr_g[:].rearrange("p a b -> p (a b)"),
                in_=xrot[:, g0 * XW:(g0 + gn) * XW])
            xr_tiles[g0] = xr_g
            g0 += gn
        w1s = const.tile([128, 128], BF16, tag="w1s")
        nc.sync.dma_start(out=w1s[:], in_=w1s_d[:, :])
        w2s = const.tile([128, 128], BF16, tag="w2s")
        nc.sync.dma_start(out=w2s[:], in_=w2s_d[:, :])
        zeros = const.tile([128, PF], BF16, tag="zeros")
        nc.vector.memset(zeros[:], 0.0)
        b1s = const.tile([128, 1], F32, tag="b1s")
        nc.sync.dma_start(out=b1s[:], in_=b1s_d[:, :])
        b2s = const.tile([128, 1], F32, tag="b2s")
        nc.sync.dma_start(out=b2s[:], in_=b2s_d[:, :])
        pws = const.tile([128, 1], F32, tag="pws")
        nc.sync.dma_start(out=pws[:], in_=pws_d[:, :])
        zcorr = const.tile([128, 1], F32, tag="zcorr")
        nc.sync.dma_start(out=zcorr[:], in_=zcorr_d[:, :])
        mask = const.tile([128, 2], F32, tag="mask")
        nc.sync.dma_start(out=mask[:], in_=mask_d[:, :])
        pb = const.tile([2, 1], F32, tag="pb")
        nc.sync.dma_start(out=pb[:], in_=pb_d[:, :])

        # per-half-pair accumulators (exp/prodS run on [128, PH] halves)
        Zs2 = small.tile([128, 2 * NPAIR], F32, tag="Zs2")
        Ss2 = small.tile([128, 2 * NPAIR], F32, tag="Ss2")


        # ---- main pipeline: per group, generate `had` (32 DVE ops with
        # 3D access patterns spanning the group), then process the
        # group's pairs; groups interleave so ACT/PE work on group g
        # overlaps DVE had-generation of group g+1 ----
        def had_ops(had_g, gs, gn):
            """One emission thunk per DVE instruction of this group's
            had-generation, so they can be interleaved (priority-wise)
            between the previous group's pairs."""
            xg = xr_tiles[gs][:, :, :]
            ops = [lambda: nc.vector.memset(had_g[:, :, NREAL:PF], 0.0)]

            def mk(d):
                if d % 2 == 0:
                    in1 = xg[:, :, d:d + 64]
                else:
                    in1 = xg[:, :, 96 + d - 1:96 + d - 1 + 64]
                return lambda: nc.vector.tensor_mul(
                    had_g[:, :, (d - 1) * 64:d * 64], xg[:, :, 0:64], in1)

            ops += [mk(d) for d in range(1, 32)]
            # d = 32: pairs (k, k+32), k in 0..31
            ops.append(lambda: nc.vector.tensor_mul(
                had_g[:, :, 1984:2016], xg[:, :, 0:32], xg[:, :, 32:64]))
            return ops

        def do_pair(i, had_g, li):
            ps1 = ps1p.tile([128, PF], F32, tag="ps1")
            for c in range(4):
                cs = slice(512 * c, 512 * (c + 1))
                nc.tensor.matmul(ps1[:, cs], w1s[:], had_g[:, li, cs],
                                 start=True, stop=True)

            # relu evac in halves so mm2 of half h can start while half
            # h+1 is still evacuating (deps are range-granular)
            # half 1 stops at NREAL: the 32 pad columns are never read
            # downstream (exp/stt skip them too), so a_sb pads stay garbage
            a_sb = ring.tile([128, PF], BF16, tag="a")
            for h in range(2):
                w = PH if h == 0 else NREAL - PH
                hs = slice(PH * h, PH * h + w)
                if i in RELU_DVE_PAIRS:
                    nc.vector.scalar_tensor_tensor(
                        out=a_sb[:, hs], in0=ps1[:, hs], scalar=b1s[:],
                        in1=zeros[:, 0:w],
                        op0=mybir.AluOpType.add, op1=mybir.AluOpType.max,
                    )
                else:
                    nc.scalar.activation(
                        a_sb[:, hs], ps1[:, hs],
                        mybir.ActivationFunctionType.Relu,
                        bias=b1s[:], scale=1.0,
                    )

            # mm2 + exp + attn-weighted-sum run on [128, PH] halves so the
            # 2-bank ps2 tiles double-buffer within 4 PSUM banks.  exp is
            # computed IN PLACE in PSUM (ScalarE->PSUM is faster than
            # ->SBUF) with the softmax normalizer via accum_out; the
            # fused product+reduce (scalar_tensor_tensor) then reads the
            # exp values straight from PSUM.
            for h in range(2):
                hs = slice(PH * h, PH * (h + 1))
                ps2 = ps2p.tile([128, PH], F32, tag="ps2")
                for c in range(2):
                    cs = slice(512 * c, 512 * (c + 1))
                    nc.tensor.matmul(ps2[:, cs], w2s[:],
                                     a_sb[:, PH * h + 512 * c:
                                          PH * h + 512 * (c + 1)],
                                     start=True, stop=True)
                # exp/stt cover only the real columns of half 1 (992 of
                # 1024): the accumulated Z then needs no dead-column
                # correction at all
                w = PH if h == 0 else NREAL - PH
                nc.scalar.activation(
                    ps2[:, 0:w], ps2[:, 0:w],
                    mybir.ActivationFunctionType.Exp,
                    bias=b2s[:], scale=1.0,
                    accum_out=Zs2[:, 2 * i + h:2 * i + h + 1],
                )
                prod = ring.tile([128, PH], BF16, tag="prod")
                nc.vector.scalar_tensor_tensor(
                    out=prod[:, 0:w],
                    in0=ps2[:, 0:w],
                    scalar=1.0,
                    in1=had_g[:, li, PH * h:PH * h + w],
                    op0=mybir.AluOpType.mult,
                    op1=mybir.AluOpType.mult,
                    accum_out=Ss2[:, 2 * i + h:2 * i + h + 1],
                )

        groups = []
        g0 = 0
        for gn in HAD_GROUPS:
            groups.append((hadp.tile([128, gn, PF], BF16, tag=f"had{g0}",
                                     name=f"had{g0}"), g0, gn))
            g0 += gn

        # group 0's had-generation runs up front; group g+1's is spread
        # across group g's pairs, emitted BEFORE each pair's own work so
        # the DVE chews on it while ACT/PE finish the pair's deps
        for op in had_ops(*groups[0]):
            op()
        for gi, (had_g, gs, gn) in enumerate(groups):
            nxt = had_ops(*groups[gi + 1]) if gi + 1 < len(groups) else []
            # spread over the first gn-1 pairs so the next group's `had`
            # is complete one pair before it's first consumed
            nslots = max(gn - 1, 1)
            per = (len(nxt) + nslots - 1) // nslots
            for li in range(gn):
                chunk, nxt = nxt[:per], nxt[per:]
                for op in chunk:
                    op()
                do_pair(gs + li, had_g, li)
            for op in nxt:
                op()

        # ---- finalize ----
        # fold half-pair accumulators: Z[:, i] = Zs2[:, 2i] + Zs2[:, 2i+1]
        Zs2v = Zs2[:].rearrange("p (i h) -> p i h", h=2)
        Ss2v = Ss2[:].rearrange("p (i h) -> p i h", h=2)
        Zs = small.tile([128, NPAIR], F32, tag="Zs")
        nc.vector.tensor_add(Zs[:], Zs2v[:, :, 0], Zs2v[:, :, 1])
        Ss = small.tile([128, NPAIR], F32, tag="Ss")
        nc.vector.tensor_add(Ss[:], Ss2v[:, :, 0], Ss2v[:, :, 1])
        # Z contains only real columns now — no dead-column correction
        rz = small.tile([128, NPAIR], F32, tag="rz")
        nc.vector.reciprocal(rz[:], Zs[:])
        v = small.tile([128, NPAIR], F32, tag="v")
        nc.vector.tensor_mul(v[:], Ss[:], rz[:])
        v2 = small.tile([128, NPAIR], F32, tag="v2")
        nc.vector.tensor_scalar_mul(v2[:], v[:], pws[:])

        # partition-halves reduction via mask matmul: [2, NPAIR]
        fin_ps = ps1p.tile([2, NPAIR], F32, tag="ps1")
        nc.tensor.matmul(fin_ps[:], mask[:], v2[:], start=True, stop=True)
        fin_sb = small.tile([2, NPAIR], F32, tag="fin")
        nc.vector.tensor_scalar(
            out=fin_sb[:], in0=fin_ps[:], scalar1=pb[:], scalar2=None,
            op0=mybir.AluOpType.add,
        )
        nc.sync.dma_start(
            out=out_d[:].rearrange("(i t) o -> t (i o)", t=2),
            in_=fin_sb[:],
        )

    nc.finalize()
    return nc


_NC = None


def _get_nc():
    global _NC
    if _NC is None:
        _NC = _build_nc()
    return _NC


def _prep_in_maps(inputs):
    bf = ml_dtypes.bfloat16
    x = np.asarray(inputs["x"], np.float32)          # [B, F, E]
    w1 = np.asarray(inputs["attn_w_w"], np.float32)  # [NHID, E]
    b1 = np.asarray(inputs["attn_w_b"], np.float32)  # [NHID]
    w2 = np.asarray(inputs["attn_h_w"], np.float32)  # [OUT, NHID]
    b2 = np.asarray(inputs["attn_h_b"], np.float32)  # [OUT]
    pw = np.asarray(inputs["attn_p_w"], np.float32)  # [1, E]
    pbv = np.asarray(inputs["attn_p_b"], np.float32) # [1]

    # block-diagonal lhsT [128, 128]: two stacked batches share the PE array
    w1s = np.zeros((128, 128), np.float32)
    w1s[0:64, 0:64] = w1.T
    w1s[64:128, 64:128] = w1.T
    w1s = w1s.astype(bf)
    w2s = np.zeros((128, 128), np.float32)
    w2s[0:64, 0:64] = w2.T
    w2s[64:128, 64:128] = w2.T
    w2s = w2s.astype(bf)
    b1s = np.tile(b1, 2).reshape(128, 1).astype(np.float32)
    b2s = np.tile(b2, 2).reshape(128, 1).astype(np.float32)
    pws = np.tile(pw[0], 2).reshape(128, 1).astype(np.float32)
    mask = np.zeros((128, 2), np.float32)
    mask[:64, 0] = 1.0
    mask[64:, 1] = 1.0
    pb2 = np.full((2, 1), float(pbv.reshape(-1)[0]), np.float32)

    # dead-column normalizer correction: dead had cols are exactly 0 =>
    # a_dead = bf16(relu(b1)); logit_dead[o] = sum_h bf16(w2)[o,h]*a_dead[h]
    # + b2[o]; each of the 32 dead cols adds exp(logit_dead) to Z.
    a_dead = np.maximum(b1, 0.0).astype(bf).astype(np.float32)       # [NHID]
    w2q = w2.astype(bf).astype(np.float32)                           # [OUT, NHID]
    logit_dead = w2q @ a_dead + b2                                   # [OUT]
    zcorr = ((PF - NREAL) * np.exp(logit_dead)).astype(np.float32)
    zcorr = np.tile(zcorr, 2).reshape(128, 1)

    idx_even = np.arange(96) % 64
    idx_odd = (np.arange(96) + 1) % 64

    common = {
        "w1s": w1s, "w2s": w2s, "b1s": b1s, "b2s": b2s,
        "pws": pws, "zcorr": zcorr, "mask": mask, "pb": pb2,
    }
    in_maps = []
    for c in range(NCORES):
        xs = x[c * BLOC:(c + 1) * BLOC]              # [64, F, E]
        xt = xs.transpose(2, 0, 1)                   # [E, b, F]
        cat = np.concatenate([xt[:, :, idx_even], xt[:, :, idx_odd]], axis=2)
        # [E, b, 192] with b = 2i + t  ->  row p = t*64 + e
        xr = (cat.reshape(E, NPAIR, 2, XW)
                 .transpose(2, 0, 1, 3)
                 .reshape(128, NPAIR * XW)
                 .astype(bf))
        in_maps.append({"xrot": np.ascontiguousarray(xr), **common})
    return in_maps


def run(inputs, trace=False):
    nc = _get_nc()
    in_maps = _prep_in_maps(inputs)
    res = run_bass_kernel_spmd(nc, in_maps, core_ids=list(range(NCORES)),
                               trace=trace)
    out = np.concatenate([res.results[c]["out"] for c in range(NCORES)], axis=0)
    return out.astype(np.float32), res


def kernel(**inputs):
    out, _ = run(inputs, trace=False)
    return out

